# revision 1
# baseline (speedup 1.0000x reference)
"""Trainium2 Bass kernel for nn_AtomEncoder (gnn_message_passing).

Strategy (8 NeuronCores, data-parallel over batch, 4 molecules/core):
  - embedding lookups: hardware dma_gather from a concatenated bf16 table
  - MLP (two residual 1x1-conv blocks + head): bf16 PE matmuls, weights
    stationary, activations kept in [d, l] orientation
  - bond aggregation: dense one-hot adjacency matmul A^T @ msg on the PE
    (A built host-side from bond indices; self-loops dropped by zeroing
    the diagonal)
  - final output emb + agg assembled in [l, d] tiles and DMA'd out
"""

import os
import sys
import types

sys.path.insert(0, "/opt/trn_rl_repo")

import numpy as np
import ml_dtypes

BF16 = ml_dtypes.bfloat16

B, L, D, K, NCORES = 32, 512, 256, 6, 8
BPC = B // NCORES          # batch elements per core
T = BPC * L                # tokens per core
NTOK = 128                 # element vocab
# concatenated table offsets: [elem(128), aroma x charge(26), seg x react(60)]
OFF_AC, OFF_SR = 128, 154
VCAT = 216                 # padded concat table rows

LAST_RESULTS = None        # BassKernelResults of the most recent run (for test.py)


def _install_ntff_hook():
    """The agent image lacks antenv.axon_hooks; synthesize it so
    run_bass_kernel_spmd(trace=True) can profile via NTFF."""
    try:
        from antenv.axon_hooks import get_axon_ntff_profile_hook  # noqa: F401
        return
    except ImportError:
        pass
    try:
        import antenv
    except ImportError:
        return
    m = types.ModuleType("antenv.axon_hooks")
    m._hook = None
    m.set_axon_ntff_profile_hook = lambda h: setattr(m, "_hook", h)
    m.get_axon_ntff_profile_hook = lambda: m._hook
    sys.modules["antenv.axon_hooks"] = m
    antenv.axon_hooks = m
    try:
        if "/root/.axon_site" not in sys.path:
            sys.path.append("/root/.axon_site")
        from trn_agent_boot.trn_boot import _ntff_profile_via_ctypes
        m._hook = _ntff_profile_via_ctypes("/opt/axon/libaxon_pjrt.so")
    except Exception:
        pass


_install_ntff_hook()

import concourse.bacc as bacc           # noqa: E402
import concourse.mybir as mybir         # noqa: E402
import concourse.bass_utils as bass_utils  # noqa: E402
from concourse.tile import TileContext  # noqa: E402
from concourse.masks import make_identity  # noqa: E402

# zero-egress container: skip artifact upload in the trace path
bass_utils.upload_artifacts = lambda tmpdir: f"local:{tmpdir}"

F32 = mybir.dt.float32
BF = mybir.dt.bfloat16
I16 = mybir.dt.int16
AF = mybir.ActivationFunctionType
ALU = mybir.AluOpType

_prog_cache = {}


def _build_program(bias_flags):
    """bias_flags: (b1,b2,b3,b4,b5) bools — emit bias matmuls only if nonzero."""
    key = tuple(bias_flags)
    if key in _prog_cache:
        return _prog_cache[key]

    nc = bacc.Bacc("TRN2", target_bir_lowering=False, debug=False,
                   num_devices=NCORES)

    gidx = nc.dram_tensor("gidx", [128, 384], I16, kind="ExternalInput")
    tabs = nc.dram_tensor("tabs", [VCAT, D], BF, kind="ExternalInput")
    ped = nc.dram_tensor("pe", [128, BPC, D], F32, kind="ExternalInput")
    w1t = nc.dram_tensor("w1t", [128, 2, 4 * D], BF, kind="ExternalInput")
    w2t = nc.dram_tensor("w2t", [128, 8, D], BF, kind="ExternalInput")
    w3t = nc.dram_tensor("w3t", [128, 2, 4 * D], BF, kind="ExternalInput")
    w4t = nc.dram_tensor("w4t", [128, 8, D], BF, kind="ExternalInput")
    w5t = nc.dram_tensor("w5t", [128, 2, D], BF, kind="ExternalInput")
    atd = nc.dram_tensor("at", [BPC, 128, 4, L], BF, kind="ExternalInput")
    idfd = nc.dram_tensor("idf", [128, 128], F32, kind="ExternalInput")
    idbd = nc.dram_tensor("idb", [128, 128], BF, kind="ExternalInput")
    biasd = []
    bdims = [4 * D, D, 4 * D, D, D]
    for i, flag in enumerate(bias_flags):
        biasd.append(
            nc.dram_tensor(f"bias{i + 1}", [1, bdims[i]], BF, kind="ExternalInput")
            if flag else None)
    outp = nc.dram_tensor("out", [L, BPC, D], F32, kind="ExternalOutput")

    with TileContext(nc) as tc:
        with (
            tc.tile_pool(name="const", bufs=1) as cpool,
            tc.tile_pool(name="apool", bufs=4) as apool,
            tc.tile_pool(name="xpool", bufs=9) as xpool,
            tc.tile_pool(name="ypool", bufs=4) as ypool,
            tc.tile_pool(name="mpool", bufs=4) as mpool,
            tc.tile_pool(name="opool", bufs=4) as opool,
            tc.tile_pool(name="epool", bufs=3) as epool,
            tc.tile_pool(name="psum", bufs=6, space="PSUM") as ppool,
            tc.tile_pool(name="psumt", bufs=2, space="PSUM") as tpool,
        ):
            # ---- identities via DMA (keep gpsimd free for gathers) ----
            idf = cpool.tile([128, 128], F32)
            nc.sync.dma_start(out=idf[:], in_=idfd[:])
            idb = cpool.tile([128, 128], BF)
            nc.sync.dma_start(out=idb[:], in_=idbd[:])

            # ---- index load + two fused gathers ----
            # b0 gather: index stream = concat_j idx[j, b0]  -> G0 chunks (j, lc)
            # rest gather: concat_j idx[j, b1..b3]           -> GR chunks (j, b-1, lc)
            idx_sb = cpool.tile([128, 384], I16)
            nc.sync.dma_start(out=idx_sb[:], in_=gidx[:])
            # HW limit: one gather must stay under the SWDGE descriptor ring
            # (~1024); 512-row gathers are proven. Same index layout as the
            # fused streams, issued as 512-token sub-blocks.
            G0 = cpool.tile([128, 12, D], BF)
            for j in range(3):
                nc.gpsimd.dma_gather(
                    out_ap=G0[:, j * 4:(j + 1) * 4, :], in_ap=tabs[:],
                    idxs_ap=idx_sb[:, j * 32:(j + 1) * 32],
                    num_idxs=L, num_idxs_reg=L, elem_size=D,
                    single_packet=False)
            GR = cpool.tile([128, 36, D], BF)
            for j in range(3):
                for bm1 in range(3):
                    co = 96 + j * 96 + bm1 * 32
                    nc.gpsimd.dma_gather(
                        out_ap=GR[:, j * 12 + bm1 * 4: j * 12 + bm1 * 4 + 4, :],
                        in_ap=tabs[:], idxs_ap=idx_sb[:, co:co + 32],
                        num_idxs=L, num_idxs_reg=L, elem_size=D,
                        single_packet=False)

            def gslice(j, b):
                """G rows of table j, molecule b -> [128, 4, D] AP."""
                if b == 0:
                    return G0[:, j * 4:(j + 1) * 4, :]
                return GR[:, j * 12 + (b - 1) * 4: j * 12 + (b - 1) * 4 + 4, :]

            # ---- resident loads ----
            pe_sb = cpool.tile([128, BPC, D], F32)
            nc.sync.dma_start(out=pe_sb[:], in_=ped[:])
            w1s = cpool.tile([128, 2, 4 * D], BF)
            nc.sync.dma_start(out=w1s[:], in_=w1t[:])
            w2s = cpool.tile([128, 8, D], BF)
            nc.sync.dma_start(out=w2s[:], in_=w2t[:])
            w3s = cpool.tile([128, 2, 4 * D], BF)
            nc.sync.dma_start(out=w3s[:], in_=w3t[:])
            w4s = cpool.tile([128, 8, D], BF)
            nc.sync.dma_start(out=w4s[:], in_=w4t[:])
            w5s = cpool.tile([128, 2, D], BF)
            nc.sync.dma_start(out=w5s[:], in_=w5t[:])
            bias_sb = []
            for i, dram in enumerate(biasd):
                if dram is None:
                    bias_sb.append(None)
                else:
                    t = cpool.tile([1, bdims[i]], BF, tag=f"bias{i}")
                    nc.sync.dma_start(out=t[:], in_=dram[:])
                    bias_sb.append(t)
            any_bias = any(bias_flags)
            if any_bias:
                ones = cpool.tile([1, L], BF)
                nc.vector.memset(ones[:], 1.0)

            # ---- per-b embedding sum (+pe) -> emb_b f32 [128, 4(lc), 256] ----
            embs = []
            for b in range(BPC):
                t1 = epool.tile([128, 4, D], BF, tag="et1")
                nc.vector.tensor_tensor(out=t1[:], in0=gslice(0, b),
                                        in1=gslice(1, b), op=ALU.add)
                t2 = epool.tile([128, 4, D], BF, tag="et2")
                nc.vector.tensor_tensor(out=t2[:], in0=t1[:], in1=gslice(2, b),
                                        op=ALU.add)
                emb_b = cpool.tile([128, 4, D], BF, tag=f"emb{b}")
                nc.vector.tensor_tensor(out=emb_b[:], in0=t2[:], in1=pe_sb[:],
                                        op=ALU.add)
                embs.append(emb_b)

            # ---- MLP: layer-major across the 4 molecules (keeps PE dense) ----
            def dense(xin, wsb, kchunks, mchunks, bsb, res=None):
                """out[m] accumulated in PSUM; returns list of psum tiles.
                res: optional residual input — PSUM pre-filled with I @ res[m]."""
                pss = []
                for m in range(mchunks):
                    ps = ppool.tile([128, L], F32, tag="mm")
                    last = kchunks - 1
                    if res is not None:
                        nc.tensor.matmul(
                            out=ps[:], lhsT=idb[:], rhs=res[:, m, :],
                            start=True, stop=False)
                    for kc in range(kchunks):
                        nc.tensor.matmul(
                            out=ps[:],
                            lhsT=wsb[:, kc, m * 128:(m + 1) * 128],
                            rhs=xin[:, kc, :],
                            start=(kc == 0 and res is None),
                            stop=(kc == last and bsb is None))
                    if bsb is not None:
                        nc.tensor.matmul(
                            out=ps[:],
                            lhsT=bsb[:1, m * 128:(m + 1) * 128],
                            rhs=ones[:1, :],
                            start=False, stop=True)
                    pss.append(ps)
                return pss

            def relu_out(dst, ps, m):
                # 3 of 8 relus on DVE, 5 on ACT
                if m % 8 in (0, 3, 6):
                    nc.vector.tensor_scalar(
                        out=dst, in0=ps[:], scalar1=0.0, scalar2=None,
                        op0=ALU.max)
                else:
                    nc.scalar.activation(out=dst, in_=ps[:], func=AF.Relu)

            # x0 = emb^T per molecule (bf16, [d partitions, l free])
            xs = []
            for b in range(BPC):
                x = xpool.tile([128, 2, L], BF, tag="x")
                for dc in range(2):
                    for lt in range(4):
                        tp = tpool.tile([128, 128], BF, tag="tp")
                        nc.tensor.transpose(
                            out=tp[:],
                            in_=embs[b][:, lt, dc * 128:(dc + 1) * 128],
                            identity=idb[:])
                        nc.scalar.activation(
                            out=x[:, dc, lt * 128:(lt + 1) * 128], in_=tp[:],
                            func=AF.Copy)
                xs.append(x)

            # L1: y1 = relu(w1 x + b1)   [1024, 512]
            y1s = []
            for b in range(BPC):
                y1 = ypool.tile([128, 8, L], BF, tag="y1")
                for m, ps in enumerate(dense(xs[b], w1s, 2, 8, bias_sb[0])):
                    relu_out(y1[:, m, :], ps, m)
                y1s.append(y1)
            # L2: x1 = x + w2 y1 + b2
            x1s = []
            for b in range(BPC):
                x1 = xpool.tile([128, 2, L], BF, tag="x")
                for m, ps in enumerate(dense(y1s[b], w2s, 8, 2, bias_sb[1])):
                    nc.vector.tensor_tensor(out=x1[:, m, :], in0=ps[:],
                                            in1=xs[b][:, m, :], op=ALU.add)
                x1s.append(x1)
            # L3: y3 = relu(w3 x1 + b3)
            y3s = []
            for b in range(BPC):
                y3 = ypool.tile([128, 8, L], BF, tag="y1")
                for m, ps in enumerate(dense(x1s[b], w3s, 2, 8, bias_sb[2])):
                    relu_out(y3[:, m, :], ps, m)
                y3s.append(y3)
            # L4: x2 = x1 + w4 y3 + b4
            x2s = []
            for b in range(BPC):
                x2 = xpool.tile([128, 2, L], BF, tag="x")
                for m, ps in enumerate(dense(y3s[b], w4s, 8, 2, bias_sb[3])):
                    nc.vector.tensor_tensor(out=x2[:, m, :], in0=ps[:],
                                            in1=x1s[b][:, m, :], op=ALU.add)
                x2s.append(x2)
            # L5: msg = w5 x2 + b5
            msgs = []
            for b in range(BPC):
                msg = mpool.tile([128, 2, L], BF, tag="msg")
                for m, ps in enumerate(dense(x2s[b], w5s, 2, 2, bias_sb[4])):
                    nc.scalar.activation(out=msg[:, m, :], in_=ps[:], func=AF.Copy)
                msgs.append(msg)

            # msg^T per molecule: [l partitions, d free] chunks
            msgTs = []
            for b in range(BPC):
                msgT = mpool.tile([128, 4, D], BF, tag="msgT")
                for dc in range(2):
                    for lt in range(4):
                        tp = tpool.tile([128, 128], BF, tag="tp")
                        nc.tensor.transpose(
                            out=tp[:], in_=msgs[b][:, dc, lt * 128:(lt + 1) * 128],
                            identity=idb[:])
                        if (dc * 4 + lt) % 2:
                            nc.scalar.activation(
                                out=msgT[:, lt, dc * 128:(dc + 1) * 128], in_=tp[:],
                                func=AF.Copy)
                        else:
                            nc.vector.tensor_copy(
                                out=msgT[:, lt, dc * 128:(dc + 1) * 128], in_=tp[:])
                msgTs.append(msgT)

            # agg + residual + store
            for b in range(BPC):
                at_sb = apool.tile([128, 4, L], BF, tag="at")
                nc.sync.dma_start(out=at_sb[:], in_=atd[b])
                for lt in range(4):
                    pa = ppool.tile([128, D], F32, tag="mm")
                    for jc in range(4):
                        nc.tensor.matmul(
                            out=pa[:],
                            lhsT=at_sb[:, jc, lt * 128:(lt + 1) * 128],
                            rhs=msgTs[b][:, jc, :],
                            start=(jc == 0), stop=(jc == 3))
                    ost = opool.tile([128, D], F32, tag="ost")
                    nc.vector.tensor_tensor(out=ost[:], in0=pa[:],
                                            in1=embs[b][:, lt, :], op=ALU.add)
                    nc.sync.dma_start(out=outp[lt * 128:(lt + 1) * 128, b, :],
                                      in_=ost[:])

    nc.compile()
    _prog_cache[key] = nc
    return nc


def _host_prep(inp):
    """Build per-core in_maps."""
    element = np.asarray(inp["element"]).astype(np.int64)
    bond = np.asarray(inp["bond"]).astype(np.int64)
    aroma = np.asarray(inp["aroma"]).astype(np.int64)
    charge = np.asarray(inp["charge"]).astype(np.int64)
    segment = np.asarray(inp["segment"]).astype(np.int64)
    react = np.asarray(inp["reactant_mask"]).astype(np.int64)

    tabs = np.zeros((VCAT, D), np.float32)
    tabs[0:128] = np.asarray(inp["elem_emb"])
    ar = np.asarray(inp["aroma_emb"], dtype=np.float32)
    ch = np.asarray(inp["charge_emb"], dtype=np.float32)
    sg = np.asarray(inp["seg_emb"], dtype=np.float32)
    rc = np.asarray(inp["react_emb"], dtype=np.float32)
    tabs[OFF_AC:OFF_AC + 26] = (ar[:, None, :] + ch[None, :, :]).reshape(26, D)
    tabs[OFF_SR:OFF_SR + 60] = (sg[:, None, :] + rc[None, :, :]).reshape(60, D)
    tabs = tabs.astype(BF16)

    pe = np.asarray(inp["pe"]).reshape(L, D).astype(np.float32)
    pe_host = np.ascontiguousarray(pe.reshape(4, 128, D).transpose(1, 0, 2))

    def wprep(w, kchunks):  # w [dout, din] -> [128, kchunks, dout] bf16
        wT = np.asarray(w).T  # [din, dout]
        return np.ascontiguousarray(
            wT.reshape(kchunks, 128, wT.shape[1]).transpose(1, 0, 2)).astype(BF16)

    w1t = wprep(inp["w1"], 2)
    w2t = wprep(inp["w2"], 8)
    w3t = wprep(inp["w3"], 2)
    w4t = wprep(inp["w4"], 8)
    w5t = wprep(inp["w5"], 2)

    biases = [np.asarray(inp[f"b{i}"]).astype(np.float32) for i in range(1, 6)]
    bias_flags = tuple(bool(np.any(b != 0.0)) for b in biases)
    bias_rows = [b.reshape(1, -1).astype(BF16) for b in biases]

    # adjusted gather indices [3, B, L]
    idx5 = np.stack([
        element,
        OFF_AC + aroma * 13 + (charge + 6),
        OFF_SR + segment * 2 + react,
    ]).astype(np.int16)

    # adjacency A^T per molecule
    lidx = np.arange(L)
    lrep = np.repeat(lidx, K)
    in_maps = []
    for c in range(NCORES):
        bs = slice(c * BPC, (c + 1) * BPC)
        # gidx [128, 640]: cols 0:160 = b0 stream (concat_j idx[j, b0]),
        # cols 160:640 = rest stream (concat_j idx[j, b1..b3]); each wrap-16
        # ([16, n/16] with token i at [i%16, i//16]), replicated x8 over
        # partition groups.
        s0 = idx5[:, c * BPC].reshape(3 * L)
        sR = idx5[:, c * BPC + 1:(c + 1) * BPC].reshape(9 * L)
        gidx = np.empty((128, 384), np.int16)
        gidx[:, 0:96] = np.tile(s0.reshape(-1, 16).T, (8, 1))
        gidx[:, 96:384] = np.tile(sR.reshape(-1, 16).T, (8, 1))
        at = np.empty((BPC, 128, 4, L), np.float32)
        for bl, bg in enumerate(range(c * BPC, (c + 1) * BPC)):
            A = np.zeros((L, L), np.float32)
            np.add.at(A, (lrep, bond[bg].ravel()), 1.0)
            A[lidx, lidx] = 0.0
            at[bl] = A.T.reshape(4, 128, L).transpose(1, 0, 2)
        m = {
            "idf": np.eye(128, dtype=np.float32),
            "idb": np.eye(128, dtype=np.float32).astype(BF16),
            "gidx": gidx,
            "tabs": tabs,
            "pe": pe_host,
            "w1t": w1t, "w2t": w2t, "w3t": w3t, "w4t": w4t, "w5t": w5t,
            "at": at.astype(BF16),
        }
        for i, flag in enumerate(bias_flags):
            if flag:
                m[f"bias{i + 1}"] = bias_rows[i]
        in_maps.append(m)
    return in_maps, bias_flags


def kernel(**inputs):
    global LAST_RESULTS
    from concourse.bass_utils import run_bass_kernel_spmd
    in_maps, bias_flags = _host_prep(inputs)
    nc = _build_program(bias_flags)
    trace = bool(int(os.environ.get("ATOM_TRACE", "0")))
    res = run_bass_kernel_spmd(nc, in_maps, list(range(NCORES)), trace=trace)
    LAST_RESULTS = res
    out = np.empty((L, B, D), np.float32)
    for c in range(NCORES):
        out[:, c * BPC:(c + 1) * BPC, :] = res.results[c]["out"]
    return out



# revision 4
# speedup vs baseline: 1.6170x; 1.6170x over previous
"""Trainium2 Bass kernel for nn_AtomEncoder (gnn_message_passing).

Strategy (8 NeuronCores, data-parallel over batch, 4 molecules/core):
  - embedding lookups as a multi-hot matmul: emb^T[d,l] = tabs^T @ MH with
    MH a host-built 0/1 matrix (3 ones per token column); pe added on the
    PSUM->SBUF move.  No gathers, no transposes, PE starts immediately.
  - MLP (two residual 1x1-conv blocks): bf16 PE matmuls in [d, l]
    orientation, weight-stationary inner ordering across molecules.
  - head computed transposed: msgT[l,d] = x2^T @ W5^T (x2 stationary).
  - bond aggregation transposed: aggT[d,l] = msgT^T @ A^T (A host-built
    dense one-hot adjacency, diagonal zeroed); out^T = aggT + x via DVE.
  - output stored [d, l] bf16 in 4 contiguous blocks, unshuffled on host.
"""

import os
import sys
import types

sys.path.insert(0, "/opt/trn_rl_repo")

import numpy as np
import ml_dtypes

BF16 = ml_dtypes.bfloat16

B, L, D, K, NCORES = 32, 512, 256, 6, 8
BPC = B // NCORES          # batch elements per core
NTOK = 128                 # element vocab
# concatenated table offsets: [elem(128), aroma x charge(26), seg x react(60)]
OFF_AC, OFF_SR = 128, 154
VCAT = 256                 # padded concat table rows (2 partition chunks)

LAST_RESULTS = None        # BassKernelResults of the most recent run (for test.py)


def _install_ntff_hook():
    """The agent image lacks antenv.axon_hooks; synthesize it so
    run_bass_kernel_spmd(trace=True) can profile via NTFF."""
    try:
        from antenv.axon_hooks import get_axon_ntff_profile_hook  # noqa: F401
        return
    except ImportError:
        pass
    try:
        import antenv
    except ImportError:
        return
    m = types.ModuleType("antenv.axon_hooks")
    m._hook = None
    m.set_axon_ntff_profile_hook = lambda h: setattr(m, "_hook", h)
    m.get_axon_ntff_profile_hook = lambda: m._hook
    sys.modules["antenv.axon_hooks"] = m
    antenv.axon_hooks = m
    try:
        if "/root/.axon_site" not in sys.path:
            sys.path.append("/root/.axon_site")
        from trn_agent_boot.trn_boot import _ntff_profile_via_ctypes
        m._hook = _ntff_profile_via_ctypes("/opt/axon/libaxon_pjrt.so")
    except Exception:
        pass


_install_ntff_hook()

import concourse.bacc as bacc           # noqa: E402
import concourse.mybir as mybir         # noqa: E402
import concourse.bass_utils as bass_utils  # noqa: E402
from concourse.tile import TileContext  # noqa: E402

# zero-egress container: skip artifact upload in the trace path
bass_utils.upload_artifacts = lambda tmpdir: f"local:{tmpdir}"

F32 = mybir.dt.float32
BF = mybir.dt.bfloat16
AF = mybir.ActivationFunctionType
ALU = mybir.AluOpType

_prog_cache = {}


def _build_program(bias_flags):
    """bias_flags: (b1,b2,b3,b4,b5) bools — emit bias matmuls only if nonzero."""
    key = tuple(bias_flags)
    if key in _prog_cache:
        return _prog_cache[key]

    nc = bacc.Bacc("TRN2", target_bir_lowering=False, debug=False,
                   num_devices=NCORES)

    tabsd = nc.dram_tensor("tabs", [128, 2, D], BF, kind="ExternalInput")
    mhd = nc.dram_tensor("mh", [BPC, 128, 2, L], BF, kind="ExternalInput")
    pedld = nc.dram_tensor("pedl", [128, 2, L], BF, kind="ExternalInput")
    w1t = nc.dram_tensor("w1t", [128, 2, 4 * D], BF, kind="ExternalInput")
    w2t = nc.dram_tensor("w2t", [128, 8, D], BF, kind="ExternalInput")
    w3t = nc.dram_tensor("w3t", [128, 2, 4 * D], BF, kind="ExternalInput")
    w4t = nc.dram_tensor("w4t", [128, 8, D], BF, kind="ExternalInput")
    w5t = nc.dram_tensor("w5t", [128, 2, D], BF, kind="ExternalInput")
    atd = nc.dram_tensor("at", [BPC, 128, 4, L], BF, kind="ExternalInput")
    biasd = []
    bdims = [4 * D, D, 4 * D, D, D]
    for i, flag in enumerate(bias_flags):
        biasd.append(
            nc.dram_tensor(f"bias{i + 1}", [1, bdims[i]], BF, kind="ExternalInput")
            if flag else None)
    outp = nc.dram_tensor("out", [BPC, 128, 2, L], BF, kind="ExternalOutput")

    with TileContext(nc) as tc:
        with (
            tc.tile_pool(name="const", bufs=1) as cpool,
            tc.tile_pool(name="apool", bufs=4) as apool,
            tc.tile_pool(name="xpool", bufs=12) as xpool,
            tc.tile_pool(name="ypool", bufs=5) as ypool,
            tc.tile_pool(name="mpool", bufs=4) as mpool,
            tc.tile_pool(name="opool", bufs=4) as opool,
            tc.tile_pool(name="psum", bufs=6, space="PSUM") as ppool,
            tc.tile_pool(name="psum5", bufs=2, space="PSUM") as p5pool,
        ):
            # ---- critical-path loads on the sync queue ----
            tabs_sb = cpool.tile([128, 2, D], BF)
            nc.sync.dma_start(out=tabs_sb[:], in_=tabsd[:])
            mh_sb = []
            for b in range(BPC):
                t = cpool.tile([128, 2, L], BF, tag=f"mh{b}")
                nc.sync.dma_start(out=t[:], in_=mhd[b])
                mh_sb.append(t)
            pedl_sb = cpool.tile([128, 2, L], BF)
            nc.sync.dma_start(out=pedl_sb[:], in_=pedld[:])

            # ---- weight / adjacency prefetch on the gpsimd queue ----
            w1s = cpool.tile([128, 2, 4 * D], BF)
            nc.gpsimd.dma_start(out=w1s[:], in_=w1t[:])
            w2s = cpool.tile([128, 8, D], BF)
            nc.gpsimd.dma_start(out=w2s[:], in_=w2t[:])
            w3s = cpool.tile([128, 2, 4 * D], BF)
            nc.gpsimd.dma_start(out=w3s[:], in_=w3t[:])
            w4s = cpool.tile([128, 8, D], BF)
            nc.gpsimd.dma_start(out=w4s[:], in_=w4t[:])
            w5s = cpool.tile([128, 2, D], BF)
            nc.gpsimd.dma_start(out=w5s[:], in_=w5t[:])
            at_sb = []
            for b in range(BPC):
                t = apool.tile([128, 4, L], BF, tag="at")
                nc.gpsimd.dma_start(out=t[:], in_=atd[b])
                at_sb.append(t)

            bias_sb = []
            for i, dram in enumerate(biasd):
                if dram is None:
                    bias_sb.append(None)
                else:
                    t = cpool.tile([1, bdims[i]], BF, tag=f"bias{i}")
                    nc.sync.dma_start(out=t[:], in_=dram[:])
                    bias_sb.append(t)
            any_bias = any(bias_flags)
            if any_bias:
                ones = cpool.tile([1, L], BF)
                nc.vector.memset(ones[:], 1.0)

            # ---- emb^T = tabs^T @ MH (+pe on the PSUM->SBUF move) ----
            # x[b] bf16 [128(d), 2(dc), 512(l)] — the MLP input AND the
            # emb term of the output (kept resident until the end).
            xs = []
            for b in range(BPC):
                x = xpool.tile([128, 2, L], BF, tag="x")
                for dc in range(2):
                    ps = ppool.tile([128, L], F32, tag="mm")
                    for vc in range(2):
                        nc.tensor.matmul(
                            out=ps[:],
                            lhsT=tabs_sb[:, vc, dc * 128:(dc + 1) * 128],
                            rhs=mh_sb[b][:, vc, :],
                            start=(vc == 0), stop=(vc == 1))
                    nc.vector.tensor_tensor(
                        out=x[:, dc, :], in0=ps[:], in1=pedl_sb[:, dc, :],
                        op=ALU.add)
                xs.append(x)

            def relu_out(dst, ps, i):
                # split relus across ACT and DVE (ACT gets the larger share)
                if i % 8 in (0, 3):
                    nc.vector.tensor_scalar(
                        out=dst, in0=ps[:], scalar1=0.0, scalar2=None,
                        op0=ALU.max)
                else:
                    nc.scalar.activation(out=dst, in_=ps[:], func=AF.Relu)

            # expand layer: y = relu(w x + b), weight-stationary over b
            def expand(xin_list, wsb, bsb, ytag):
                youts = [ypool.tile([128, 8, L], BF, tag=ytag, name=f"y{b}")
                         for b in range(BPC)]
                for m in range(8):
                    pss = []
                    for b in range(BPC):
                        ps = ppool.tile([128, L], F32, tag="mm")
                        for kc in range(2):
                            nc.tensor.matmul(
                                out=ps[:],
                                lhsT=wsb[:, kc, m * 128:(m + 1) * 128],
                                rhs=xin_list[b][:, kc, :],
                                start=(kc == 0),
                                stop=(kc == 1 and bsb is None))
                        if bsb is not None:
                            nc.tensor.matmul(
                                out=ps[:],
                                lhsT=bsb[:1, m * 128:(m + 1) * 128],
                                rhs=ones[:1, :],
                                start=False, stop=True)
                        pss.append(ps)
                    for b in range(BPC):
                        relu_out(youts[b][:, m, :], pss[b], m * BPC + b)
                return youts

            # contract layer: xnew = xres + w y + b, weight-stationary over b
            def contract(y_list, wsb, bsb, xres_list):
                xouts = [xpool.tile([128, 2, L], BF, tag="x", name=f"xn{b}")
                         for b in range(BPC)]
                for m in range(2):
                    pss = []
                    for b in range(BPC):
                        ps = ppool.tile([128, L], F32, tag="mm")
                        for kc in range(8):
                            nc.tensor.matmul(
                                out=ps[:],
                                lhsT=wsb[:, kc, m * 128:(m + 1) * 128],
                                rhs=y_list[b][:, kc, :],
                                start=(kc == 0),
                                stop=(kc == 7 and bsb is None))
                        if bsb is not None:
                            nc.tensor.matmul(
                                out=ps[:],
                                lhsT=bsb[:1, m * 128:(m + 1) * 128],
                                rhs=ones[:1, :],
                                start=False, stop=True)
                        pss.append(ps)
                    for b in range(BPC):
                        nc.vector.tensor_tensor(
                            out=xouts[b][:, m, :], in0=pss[b][:],
                            in1=xres_list[b][:, m, :], op=ALU.add)
                return xouts

            y1s = expand(xs, w1s, bias_sb[0], "y")
            x1s = contract(y1s, w2s, bias_sb[1], xs)
            y3s = expand(x1s, w3s, bias_sb[2], "y")
            x2s = contract(y3s, w4s, bias_sb[3], x1s)

            # head (transposed): msgT[l, d] = x2^T @ W5^T; then
            # aggT[d, l] = msgT^T @ A^T; out^T = aggT + x.
            def head(b):
                msgT = mpool.tile([128, 4, D], BF, tag="msgT")
                for jc in range(4):
                    ps = p5pool.tile([128, D], F32, tag="p5")
                    for dc in range(2):
                        nc.tensor.matmul(
                            out=ps[:],
                            lhsT=x2s[b][:, dc, jc * 128:(jc + 1) * 128],
                            rhs=w5s[:, dc, :],
                            start=(dc == 0),
                            stop=(dc == 1 and bias_sb[4] is None))
                    if bias_sb[4] is not None:
                        nc.tensor.matmul(
                            out=ps[:],
                            lhsT=ones[:1, jc * 128:(jc + 1) * 128],
                            rhs=bias_sb[4][:1, :],
                            start=False, stop=True)
                    nc.scalar.activation(out=msgT[:, jc, :], in_=ps[:],
                                         func=AF.Copy)
                return msgT

            def agg(b, msgT):
                ot = opool.tile([128, 2, L], BF, tag="ot")
                for dc in range(2):
                    ps = ppool.tile([128, L], F32, tag="mm")
                    for jc in range(4):
                        nc.tensor.matmul(
                            out=ps[:],
                            lhsT=msgT[:, jc, dc * 128:(dc + 1) * 128],
                            rhs=at_sb[b][:, jc, :],
                            start=(jc == 0), stop=(jc == 3))
                    nc.vector.tensor_tensor(
                        out=ot[:, dc, :], in0=ps[:], in1=xs[b][:, dc, :],
                        op=ALU.add)
                nc.sync.dma_start(out=outp[b], in_=ot[:])

            # software-pipeline the head: msgT[b+1] copies overlap aggT[b]
            msgTs = [head(0), head(1)]
            agg(0, msgTs[0])
            msgTs.append(head(2))
            agg(1, msgTs[1])
            msgTs.append(head(3))
            agg(2, msgTs[2])
            agg(3, msgTs[3])

    nc.compile()
    _prog_cache[key] = nc
    return nc


def _host_prep(inp):
    """Build per-core in_maps."""
    element = np.asarray(inp["element"]).astype(np.int64)
    bond = np.asarray(inp["bond"]).astype(np.int64)
    aroma = np.asarray(inp["aroma"]).astype(np.int64)
    charge = np.asarray(inp["charge"]).astype(np.int64)
    segment = np.asarray(inp["segment"]).astype(np.int64)
    react = np.asarray(inp["reactant_mask"]).astype(np.int64)

    tab = np.zeros((VCAT, D), np.float32)
    tab[0:128] = np.asarray(inp["elem_emb"])
    ar = np.asarray(inp["aroma_emb"], dtype=np.float32)
    ch = np.asarray(inp["charge_emb"], dtype=np.float32)
    sg = np.asarray(inp["seg_emb"], dtype=np.float32)
    rc = np.asarray(inp["react_emb"], dtype=np.float32)
    tab[OFF_AC:OFF_AC + 26] = (ar[:, None, :] + ch[None, :, :]).reshape(26, D)
    tab[OFF_SR:OFF_SR + 60] = (sg[:, None, :] + rc[None, :, :]).reshape(60, D)
    tabs_host = np.ascontiguousarray(
        tab.reshape(2, 128, D).transpose(1, 0, 2)).astype(BF16)

    pe = np.asarray(inp["pe"]).reshape(L, D).astype(np.float32)
    pedl_host = np.ascontiguousarray(
        pe.T.reshape(2, 128, L).transpose(1, 0, 2)).astype(BF16)

    def wprep(w, kchunks):  # w [dout, din] -> [128, kchunks, dout] bf16
        wT = np.asarray(w).T  # [din, dout]
        return np.ascontiguousarray(
            wT.reshape(kchunks, 128, wT.shape[1]).transpose(1, 0, 2)).astype(BF16)

    w1t = wprep(inp["w1"], 2)
    w2t = wprep(inp["w2"], 8)
    w3t = wprep(inp["w3"], 2)
    w4t = wprep(inp["w4"], 8)
    w5t = wprep(inp["w5"], 2)

    biases = [np.asarray(inp[f"b{i}"]).astype(np.float32) for i in range(1, 6)]
    bias_flags = tuple(bool(np.any(b != 0.0)) for b in biases)
    bias_rows = [b.reshape(1, -1).astype(BF16) for b in biases]

    # multi-hot [B, VCAT, L]: 3 disjoint one rows per token column
    rows = np.stack([
        element,
        OFF_AC + aroma * 13 + (charge + 6),
        OFF_SR + segment * 2 + react,
    ])  # [3, B, L]
    mh = np.zeros((B, VCAT, L), np.float32)
    bidx = np.arange(B)[None, :, None]
    lidx = np.arange(L)[None, None, :]
    mh[bidx, rows, lidx] = 1.0
    mh_host = np.ascontiguousarray(
        mh.reshape(B, 2, 128, L).transpose(0, 2, 1, 3)).astype(BF16)

    # adjacency A^T per molecule
    lr = np.arange(L)
    lrep = np.repeat(lr, K)
    in_maps = []
    for c in range(NCORES):
        at = np.empty((BPC, 128, 4, L), np.float32)
        for bl, bg in enumerate(range(c * BPC, (c + 1) * BPC)):
            A = np.zeros((L, L), np.float32)
            np.add.at(A, (lrep, bond[bg].ravel()), 1.0)
            A[lr, lr] = 0.0
            at[bl] = A.T.reshape(4, 128, L).transpose(1, 0, 2)
        m = {
            "tabs": tabs_host,
            "mh": mh_host[c * BPC:(c + 1) * BPC],
            "pedl": pedl_host,
            "w1t": w1t, "w2t": w2t, "w3t": w3t, "w4t": w4t, "w5t": w5t,
            "at": at.astype(BF16),
        }
        for i, flag in enumerate(bias_flags):
            if flag:
                m[f"bias{i + 1}"] = bias_rows[i]
        in_maps.append(m)
    return in_maps, bias_flags


def kernel(**inputs):
    global LAST_RESULTS
    from concourse.bass_utils import run_bass_kernel_spmd
    in_maps, bias_flags = _host_prep(inputs)
    nc = _build_program(bias_flags)
    trace = bool(int(os.environ.get("ATOM_TRACE", "0")))
    res = run_bass_kernel_spmd(nc, in_maps, list(range(NCORES)), trace=trace)
    LAST_RESULTS = res
    out = np.empty((L, B, D), np.float32)
    for c in range(NCORES):
        o = np.asarray(res.results[c]["out"]).astype(np.float32)
        # o[b, p, dc, l] -> out[l, b, dc*128+p]
        out[:, c * BPC:(c + 1) * BPC, :] = o.transpose(3, 0, 2, 1).reshape(
            L, BPC, D)
    return out


# revision 8
# speedup vs baseline: 1.7133x; 1.0596x over previous
"""Trainium2 Bass kernel for nn_AtomEncoder (gnn_message_passing).

Strategy (8 NeuronCores, data-parallel over batch, 4 molecules/core):
  - embedding lookups as a multi-hot matmul: emb^T[d,l] = tabs^T @ MH with
    MH a host-built 0/1 matrix (3 ones per token column); pe added on the
    PSUM->SBUF move.  No gathers, no transposes, PE starts immediately.
  - MLP (two residual 1x1-conv blocks): bf16 PE matmuls in [d, l]
    orientation, weight-stationary inner ordering across molecules.
  - head computed transposed: msgT[l,d] = x2^T @ W5^T (x2 stationary).
  - bond aggregation transposed: aggT[d,l] = msgT^T @ A^T (A host-built
    dense one-hot adjacency, diagonal zeroed); out^T = aggT + x via DVE.
  - output stored [d, l] bf16 in 4 contiguous blocks, unshuffled on host.
"""

import os
import sys
import types

sys.path.insert(0, "/opt/trn_rl_repo")

import numpy as np
import ml_dtypes

BF16 = ml_dtypes.bfloat16

B, L, D, K, NCORES = 32, 512, 256, 6, 8
BPC = B // NCORES          # batch elements per core
NTOK = 128                 # element vocab
# concatenated table offsets: [elem(128), aroma x charge(26), seg x react(60)]
OFF_AC, OFF_SR = 128, 154
VCAT = 256                 # padded concat table rows (2 partition chunks)

LAST_RESULTS = None        # BassKernelResults of the most recent run (for test.py)


def _install_ntff_hook():
    """The agent image lacks antenv.axon_hooks; synthesize it so
    run_bass_kernel_spmd(trace=True) can profile via NTFF."""
    try:
        from antenv.axon_hooks import get_axon_ntff_profile_hook  # noqa: F401
        return
    except ImportError:
        pass
    try:
        import antenv
    except ImportError:
        return
    m = types.ModuleType("antenv.axon_hooks")
    m._hook = None
    m.set_axon_ntff_profile_hook = lambda h: setattr(m, "_hook", h)
    m.get_axon_ntff_profile_hook = lambda: m._hook
    sys.modules["antenv.axon_hooks"] = m
    antenv.axon_hooks = m
    try:
        if "/root/.axon_site" not in sys.path:
            sys.path.append("/root/.axon_site")
        from trn_agent_boot.trn_boot import _ntff_profile_via_ctypes
        m._hook = _ntff_profile_via_ctypes("/opt/axon/libaxon_pjrt.so")
    except Exception:
        pass


_install_ntff_hook()

import concourse.bacc as bacc           # noqa: E402
import concourse.mybir as mybir         # noqa: E402
import concourse.bass_utils as bass_utils  # noqa: E402
from concourse.tile import TileContext  # noqa: E402

# zero-egress container: skip artifact upload in the trace path
bass_utils.upload_artifacts = lambda tmpdir: f"local:{tmpdir}"

F32 = mybir.dt.float32
BF = mybir.dt.bfloat16
AF = mybir.ActivationFunctionType
ALU = mybir.AluOpType

_prog_cache = {}


def _build_program(bias_flags):
    """bias_flags: (b1,b2,b3,b4,b5) bools — emit bias matmuls only if nonzero."""
    key = tuple(bias_flags)
    if key in _prog_cache:
        return _prog_cache[key]

    nc = bacc.Bacc("TRN2", target_bir_lowering=False, debug=False,
                   num_devices=NCORES)

    tabsd = nc.dram_tensor("tabs", [128, 2, D], BF, kind="ExternalInput")
    mhd = nc.dram_tensor("mh", [BPC, 128, 2, L], BF, kind="ExternalInput")
    pedld = nc.dram_tensor("pedl", [128, 2, L], BF, kind="ExternalInput")
    w1t = nc.dram_tensor("w1t", [128, 2, 4 * D], BF, kind="ExternalInput")
    w2t = nc.dram_tensor("w2t", [128, 8, D], BF, kind="ExternalInput")
    w3t = nc.dram_tensor("w3t", [128, 2, 4 * D], BF, kind="ExternalInput")
    w4t = nc.dram_tensor("w4t", [128, 8, D], BF, kind="ExternalInput")
    w5t = nc.dram_tensor("w5t", [128, 2, D], BF, kind="ExternalInput")
    atd = nc.dram_tensor("at", [BPC, 128, 4, L], BF, kind="ExternalInput")
    biasd = []
    bdims = [4 * D, D, 4 * D, D, D]
    for i, flag in enumerate(bias_flags):
        biasd.append(
            nc.dram_tensor(f"bias{i + 1}", [1, bdims[i]], BF, kind="ExternalInput")
            if flag else None)
    outp = nc.dram_tensor("out", [BPC, 128, 2, L], BF, kind="ExternalOutput")

    with TileContext(nc) as tc:
        with (
            tc.tile_pool(name="const", bufs=1) as cpool,
            tc.tile_pool(name="apool", bufs=4) as apool,
            tc.tile_pool(name="xpool", bufs=12) as xpool,
            tc.tile_pool(name="ypool", bufs=5) as ypool,
            tc.tile_pool(name="mpool", bufs=4) as mpool,
            tc.tile_pool(name="opool", bufs=4) as opool,
            tc.tile_pool(name="psum", bufs=6, space="PSUM") as ppool,
            tc.tile_pool(name="psum5", bufs=2, space="PSUM") as p5pool,
        ):
            # ---- loads in need-order on the two HW-DGE queues ----
            # sync: tabs, mh0, pedl, mh1-3, at0-3 (FIFO = priority order);
            # scalar: w1..w5 in parallel.  Software DGE (gpsimd) unused —
            # it contends for HBM and adds a long queue-drain tail.
            tabs_sb = cpool.tile([128, 2, D], BF)
            nc.sync.dma_start(out=tabs_sb[:], in_=tabsd[:])
            mh_sb = [cpool.tile([128, 2, L], BF, tag=f"mh{b}", name=f"mh{b}")
                     for b in range(BPC)]
            nc.sync.dma_start(out=mh_sb[0][:], in_=mhd[0])
            pedl_sb = cpool.tile([128, 2, L], BF)
            nc.sync.dma_start(out=pedl_sb[:], in_=pedld[:])
            for b in range(1, BPC):
                nc.sync.dma_start(out=mh_sb[b][:], in_=mhd[b])

            w1s = cpool.tile([128, 2, 4 * D], BF)
            nc.scalar.dma_start(out=w1s[:], in_=w1t[:])
            w2s = cpool.tile([128, 8, D], BF)
            nc.scalar.dma_start(out=w2s[:], in_=w2t[:])
            w3s = cpool.tile([128, 2, 4 * D], BF)
            nc.scalar.dma_start(out=w3s[:], in_=w3t[:])
            w4s = cpool.tile([128, 8, D], BF)
            nc.scalar.dma_start(out=w4s[:], in_=w4t[:])
            w5s = cpool.tile([128, 2, D], BF)
            nc.scalar.dma_start(out=w5s[:], in_=w5t[:])

            at_sb = []
            for b in range(BPC):
                t = apool.tile([128, 4, L], BF, tag="at", name=f"at{b}")
                nc.sync.dma_start(out=t[:], in_=atd[b])
                at_sb.append(t)

            bias_sb = []
            for i, dram in enumerate(biasd):
                if dram is None:
                    bias_sb.append(None)
                else:
                    t = cpool.tile([1, bdims[i]], BF, tag=f"bias{i}")
                    nc.scalar.dma_start(out=t[:], in_=dram[:])
                    bias_sb.append(t)
            any_bias = any(bias_flags)
            if any_bias:
                ones = cpool.tile([1, L], BF)
                nc.vector.memset(ones[:], 1.0)

            # ---- PE p-state warmup: dead matmuls while first DMAs land ----
            wu = cpool.tile([128, L], BF)
            nc.vector.memset(wu[:], 0.0)
            wups = ppool.tile([128, L], F32, tag="mm")
            for _ in range(6):
                nc.tensor.matmul(out=wups[:], lhsT=wu[:, 0:128], rhs=wu[:],
                                 start=True, stop=True)

            # ---- emb^T = tabs^T @ MH (+pe on the PSUM->SBUF move) ----
            # x[b] bf16 [128(d), 2(dc), 512(l)] — the MLP input AND the
            # emb term of the output (kept resident until the end).
            xs = []
            for b in range(BPC):
                x = xpool.tile([128, 2, L], BF, tag="x")
                for dc in range(2):
                    ps = ppool.tile([128, L], F32, tag="mm")
                    for vc in range(2):
                        nc.tensor.matmul(
                            out=ps[:],
                            lhsT=tabs_sb[:, vc, dc * 128:(dc + 1) * 128],
                            rhs=mh_sb[b][:, vc, :],
                            start=(vc == 0), stop=(vc == 1))
                    nc.vector.tensor_tensor(
                        out=x[:, dc, :], in0=ps[:], in1=pedl_sb[:, dc, :],
                        op=ALU.add)
                xs.append(x)

            def relu_out(dst, ps, i):
                # split relus across ACT and DVE (ACT gets the larger share)
                if i % 8 in (0, 3):
                    nc.vector.tensor_scalar(
                        out=dst, in0=ps[:], scalar1=0.0, scalar2=None,
                        op0=ALU.max)
                else:
                    nc.scalar.activation(out=dst, in_=ps[:], func=AF.Relu)

            # expand layer: y = relu(w x + b), weight-stationary over b
            def expand(xin_list, wsb, bsb, ytag):
                youts = [ypool.tile([128, 8, L], BF, tag=ytag, name=f"y{b}")
                         for b in range(BPC)]
                for m in range(8):
                    pss = []
                    for b in range(BPC):
                        ps = ppool.tile([128, L], F32, tag="mm")
                        for kc in range(2):
                            nc.tensor.matmul(
                                out=ps[:],
                                lhsT=wsb[:, kc, m * 128:(m + 1) * 128],
                                rhs=xin_list[b][:, kc, :],
                                start=(kc == 0),
                                stop=(kc == 1 and bsb is None))
                        if bsb is not None:
                            nc.tensor.matmul(
                                out=ps[:],
                                lhsT=bsb[:1, m * 128:(m + 1) * 128],
                                rhs=ones[:1, :],
                                start=False, stop=True)
                        pss.append(ps)
                    for b in range(BPC):
                        relu_out(youts[b][:, m, :], pss[b], m * BPC + b)
                return youts

            # contract layer: xnew = xres + w y + b, weight-stationary over b
            def contract(y_list, wsb, bsb, xres_list):
                xouts = [xpool.tile([128, 2, L], BF, tag="x", name=f"xn{b}")
                         for b in range(BPC)]
                for m in range(2):
                    pss = []
                    for b in range(BPC):
                        ps = ppool.tile([128, L], F32, tag="mm")
                        for kc in range(8):
                            nc.tensor.matmul(
                                out=ps[:],
                                lhsT=wsb[:, kc, m * 128:(m + 1) * 128],
                                rhs=y_list[b][:, kc, :],
                                start=(kc == 0),
                                stop=(kc == 7 and bsb is None))
                        if bsb is not None:
                            nc.tensor.matmul(
                                out=ps[:],
                                lhsT=bsb[:1, m * 128:(m + 1) * 128],
                                rhs=ones[:1, :],
                                start=False, stop=True)
                        pss.append(ps)
                    for b in range(BPC):
                        nc.vector.tensor_tensor(
                            out=xouts[b][:, m, :], in0=pss[b][:],
                            in1=xres_list[b][:, m, :], op=ALU.add)
                return xouts

            y1s = expand(xs, w1s, bias_sb[0], "y")
            x1s = contract(y1s, w2s, bias_sb[1], xs)
            y3s = expand(x1s, w3s, bias_sb[2], "y")
            x2s = contract(y3s, w4s, bias_sb[3], x1s)

            # head (transposed): msgT[l, d] = x2^T @ W5^T; then
            # aggT[d, l] = msgT^T @ A^T; out^T = aggT + x.
            def head(b):
                msgT = mpool.tile([128, 4, D], BF, tag="msgT")
                for jc in range(4):
                    ps = p5pool.tile([128, D], F32, tag="p5")
                    for dc in range(2):
                        nc.tensor.matmul(
                            out=ps[:],
                            lhsT=x2s[b][:, dc, jc * 128:(jc + 1) * 128],
                            rhs=w5s[:, dc, :],
                            start=(dc == 0),
                            stop=(dc == 1 and bias_sb[4] is None))
                    if bias_sb[4] is not None:
                        nc.tensor.matmul(
                            out=ps[:],
                            lhsT=ones[:1, jc * 128:(jc + 1) * 128],
                            rhs=bias_sb[4][:1, :],
                            start=False, stop=True)
                    if jc % 2:
                        nc.vector.tensor_copy(out=msgT[:, jc, :], in_=ps[:])
                    else:
                        nc.scalar.activation(out=msgT[:, jc, :], in_=ps[:],
                                             func=AF.Copy)
                return msgT

            def agg(b, msgT):
                ot = opool.tile([128, 2, L], BF, tag="ot")
                for dc in range(2):
                    ps = ppool.tile([128, L], F32, tag="mm")
                    for jc in range(4):
                        nc.tensor.matmul(
                            out=ps[:],
                            lhsT=msgT[:, jc, dc * 128:(dc + 1) * 128],
                            rhs=at_sb[b][:, jc, :],
                            start=(jc == 0), stop=(jc == 3))
                    nc.vector.tensor_tensor(
                        out=ot[:, dc, :], in0=ps[:], in1=xs[b][:, dc, :],
                        op=ALU.add)
                nc.sync.dma_start(out=outp[b], in_=ot[:])

            # software-pipeline the head: msgT[b+1] copies overlap aggT[b]
            msgTs = [head(0), head(1)]
            agg(0, msgTs[0])
            msgTs.append(head(2))
            agg(1, msgTs[1])
            msgTs.append(head(3))
            agg(2, msgTs[2])
            agg(3, msgTs[3])

    nc.compile()
    _prog_cache[key] = nc
    return nc


def _host_prep(inp):
    """Build per-core in_maps."""
    element = np.asarray(inp["element"]).astype(np.int64)
    bond = np.asarray(inp["bond"]).astype(np.int64)
    aroma = np.asarray(inp["aroma"]).astype(np.int64)
    charge = np.asarray(inp["charge"]).astype(np.int64)
    segment = np.asarray(inp["segment"]).astype(np.int64)
    react = np.asarray(inp["reactant_mask"]).astype(np.int64)

    tab = np.zeros((VCAT, D), np.float32)
    tab[0:128] = np.asarray(inp["elem_emb"])
    ar = np.asarray(inp["aroma_emb"], dtype=np.float32)
    ch = np.asarray(inp["charge_emb"], dtype=np.float32)
    sg = np.asarray(inp["seg_emb"], dtype=np.float32)
    rc = np.asarray(inp["react_emb"], dtype=np.float32)
    tab[OFF_AC:OFF_AC + 26] = (ar[:, None, :] + ch[None, :, :]).reshape(26, D)
    tab[OFF_SR:OFF_SR + 60] = (sg[:, None, :] + rc[None, :, :]).reshape(60, D)
    tabs_host = np.ascontiguousarray(
        tab.reshape(2, 128, D).transpose(1, 0, 2)).astype(BF16)

    pe = np.asarray(inp["pe"]).reshape(L, D).astype(np.float32)
    pedl_host = np.ascontiguousarray(
        pe.T.reshape(2, 128, L).transpose(1, 0, 2)).astype(BF16)

    def wprep(w, kchunks):  # w [dout, din] -> [128, kchunks, dout] bf16
        wT = np.asarray(w).T  # [din, dout]
        return np.ascontiguousarray(
            wT.reshape(kchunks, 128, wT.shape[1]).transpose(1, 0, 2)).astype(BF16)

    w1t = wprep(inp["w1"], 2)
    w2t = wprep(inp["w2"], 8)
    w3t = wprep(inp["w3"], 2)
    w4t = wprep(inp["w4"], 8)
    w5t = wprep(inp["w5"], 2)

    biases = [np.asarray(inp[f"b{i}"]).astype(np.float32) for i in range(1, 6)]
    bias_flags = tuple(bool(np.any(b != 0.0)) for b in biases)
    bias_rows = [b.reshape(1, -1).astype(BF16) for b in biases]

    # multi-hot [B, VCAT, L]: 3 disjoint one rows per token column
    rows = np.stack([
        element,
        OFF_AC + aroma * 13 + (charge + 6),
        OFF_SR + segment * 2 + react,
    ])  # [3, B, L]
    mh = np.zeros((B, VCAT, L), np.float32)
    bidx = np.arange(B)[None, :, None]
    lidx = np.arange(L)[None, None, :]
    mh[bidx, rows, lidx] = 1.0
    mh_host = np.ascontiguousarray(
        mh.reshape(B, 2, 128, L).transpose(0, 2, 1, 3)).astype(BF16)

    # adjacency A^T per molecule
    lr = np.arange(L)
    lrep = np.repeat(lr, K)
    in_maps = []
    for c in range(NCORES):
        at = np.empty((BPC, 128, 4, L), np.float32)
        for bl, bg in enumerate(range(c * BPC, (c + 1) * BPC)):
            A = np.zeros((L, L), np.float32)
            np.add.at(A, (lrep, bond[bg].ravel()), 1.0)
            A[lr, lr] = 0.0
            at[bl] = A.T.reshape(4, 128, L).transpose(1, 0, 2)
        m = {
            "tabs": tabs_host,
            "mh": mh_host[c * BPC:(c + 1) * BPC],
            "pedl": pedl_host,
            "w1t": w1t, "w2t": w2t, "w3t": w3t, "w4t": w4t, "w5t": w5t,
            "at": at.astype(BF16),
        }
        for i, flag in enumerate(bias_flags):
            if flag:
                m[f"bias{i + 1}"] = bias_rows[i]
        in_maps.append(m)
    return in_maps, bias_flags


def kernel(**inputs):
    global LAST_RESULTS
    from concourse.bass_utils import run_bass_kernel_spmd
    in_maps, bias_flags = _host_prep(inputs)
    nc = _build_program(bias_flags)
    trace = bool(int(os.environ.get("ATOM_TRACE", "0")))
    res = run_bass_kernel_spmd(nc, in_maps, list(range(NCORES)), trace=trace)
    LAST_RESULTS = res
    out = np.empty((L, B, D), np.float32)
    for c in range(NCORES):
        o = np.asarray(res.results[c]["out"]).astype(np.float32)
        # o[b, p, dc, l] -> out[l, b, dc*128+p]
        out[:, c * BPC:(c + 1) * BPC, :] = o.transpose(3, 0, 2, 1).reshape(
            L, BPC, D)
    return out


# revision 11
# speedup vs baseline: 1.7742x; 1.0355x over previous
"""Trainium2 Bass kernel for nn_AtomEncoder (gnn_message_passing).

Strategy (8 NeuronCores, data-parallel over batch, 4 molecules/core):
  - embedding lookups as a multi-hot matmul: emb^T[d,l] = tabs^T @ MH with
    MH a host-built 0/1 matrix (3 ones per token column); pe added on the
    PSUM->SBUF move.  No gathers, no transposes, PE starts immediately.
  - MLP (two residual 1x1-conv blocks): bf16 PE matmuls in [d, l]
    orientation, weight-stationary inner ordering across molecules.
  - head computed transposed: msgT[l,d] = x2^T @ W5^T (x2 stationary).
  - bond aggregation transposed: aggT[d,l] = msgT^T @ A^T (A host-built
    dense one-hot adjacency, diagonal zeroed); out^T = aggT + x via DVE.
  - output stored [d, l] bf16 in 4 contiguous blocks, unshuffled on host.
"""

import os
import sys
import types

sys.path.insert(0, "/opt/trn_rl_repo")

import numpy as np
import ml_dtypes

BF16 = ml_dtypes.bfloat16

B, L, D, K, NCORES = 32, 512, 256, 6, 8
BPC = B // NCORES          # batch elements per core
NTOK = 128                 # element vocab
# concatenated table offsets: [elem(128), aroma x charge(26), seg x react(60)]
OFF_AC, OFF_SR = 128, 154
VCAT = 256                 # padded concat table rows (2 partition chunks)

LAST_RESULTS = None        # BassKernelResults of the most recent run (for test.py)


def _install_ntff_hook():
    """The agent image lacks antenv.axon_hooks; synthesize it so
    run_bass_kernel_spmd(trace=True) can profile via NTFF."""
    try:
        from antenv.axon_hooks import get_axon_ntff_profile_hook  # noqa: F401
        return
    except ImportError:
        pass
    try:
        import antenv
    except ImportError:
        return
    m = types.ModuleType("antenv.axon_hooks")
    m._hook = None
    m.set_axon_ntff_profile_hook = lambda h: setattr(m, "_hook", h)
    m.get_axon_ntff_profile_hook = lambda: m._hook
    sys.modules["antenv.axon_hooks"] = m
    antenv.axon_hooks = m
    try:
        if "/root/.axon_site" not in sys.path:
            sys.path.append("/root/.axon_site")
        from trn_agent_boot.trn_boot import _ntff_profile_via_ctypes
        m._hook = _ntff_profile_via_ctypes("/opt/axon/libaxon_pjrt.so")
    except Exception:
        pass


_install_ntff_hook()

import concourse.bacc as bacc           # noqa: E402
import concourse.mybir as mybir         # noqa: E402
import concourse.bass_utils as bass_utils  # noqa: E402
from concourse.tile import TileContext  # noqa: E402

# zero-egress container: skip artifact upload in the trace path
bass_utils.upload_artifacts = lambda tmpdir: f"local:{tmpdir}"

F32 = mybir.dt.float32
BF = mybir.dt.bfloat16
AF = mybir.ActivationFunctionType
ALU = mybir.AluOpType

_prog_cache = {}


def _build_program(bias_flags):
    """bias_flags: (b1,b2,b3,b4,b5) bools — emit bias matmuls only if nonzero."""
    key = tuple(bias_flags)
    if key in _prog_cache:
        return _prog_cache[key]

    nc = bacc.Bacc("TRN2", target_bir_lowering=False, debug=False,
                   num_devices=NCORES)

    tabsd = nc.dram_tensor("tabs", [128, 2, D], BF, kind="ExternalInput")
    mhd = nc.dram_tensor("mh", [BPC, 128, 2, L], BF, kind="ExternalInput")
    pedld = nc.dram_tensor("pedl", [128, 2, L], BF, kind="ExternalInput")
    w1t = nc.dram_tensor("w1t", [128, 2, 4 * D], BF, kind="ExternalInput")
    w2t = nc.dram_tensor("w2t", [128, 8, D], BF, kind="ExternalInput")
    w3t = nc.dram_tensor("w3t", [128, 2, 4 * D], BF, kind="ExternalInput")
    w4t = nc.dram_tensor("w4t", [128, 8, D], BF, kind="ExternalInput")
    w5t = nc.dram_tensor("w5t", [128, 2, D], BF, kind="ExternalInput")
    atd = nc.dram_tensor("at", [BPC, 128, 4, L], BF, kind="ExternalInput")
    biasd = []
    bdims = [4 * D, D, 4 * D, D, D]
    for i, flag in enumerate(bias_flags):
        biasd.append(
            nc.dram_tensor(f"bias{i + 1}", [1, bdims[i]], BF, kind="ExternalInput")
            if flag else None)
    outp = nc.dram_tensor("out", [BPC, 2, 128, L], BF, kind="ExternalOutput")

    with TileContext(nc) as tc:
        with (
            tc.tile_pool(name="const", bufs=1) as cpool,
            tc.tile_pool(name="apool", bufs=4) as apool,
            tc.tile_pool(name="xpool", bufs=12) as xpool,
            tc.tile_pool(name="ypool", bufs=5) as ypool,
            tc.tile_pool(name="mpool", bufs=4) as mpool,
            tc.tile_pool(name="opool", bufs=4) as opool,
            tc.tile_pool(name="psum", bufs=6, space="PSUM") as ppool,
            tc.tile_pool(name="psum5", bufs=2, space="PSUM") as p5pool,
        ):
            # ---- loads in need-order on the two HW-DGE queues ----
            # Each queue is FIFO (priority = program order) and the two
            # queues split HBM bandwidth, so balance bytes and interleave
            # by first-use time.  Software DGE (gpsimd) unused — it
            # contends for HBM and adds a long queue-drain tail.
            # sync:   tabs, mh0, mh2, mh3, at0-3      (~2.7 MB)
            # scalar: mh1, pedl, w1..w5, biases       (~2.7 MB)
            tabs_sb = cpool.tile([128, 2, D], BF)
            nc.sync.dma_start(out=tabs_sb[:], in_=tabsd[:])
            mh_sb = [cpool.tile([128, 2, L], BF, tag=f"mh{b}", name=f"mh{b}")
                     for b in range(BPC)]
            nc.sync.dma_start(out=mh_sb[0][:], in_=mhd[0])
            nc.scalar.dma_start(out=mh_sb[1][:], in_=mhd[1])
            pedl_sb = cpool.tile([128, 2, L], BF)
            nc.scalar.dma_start(out=pedl_sb[:], in_=pedld[:])
            nc.sync.dma_start(out=mh_sb[2][:], in_=mhd[2])
            nc.sync.dma_start(out=mh_sb[3][:], in_=mhd[3])

            w1s = cpool.tile([128, 2, 4 * D], BF)
            nc.scalar.dma_start(out=w1s[:], in_=w1t[:])
            w2s = cpool.tile([128, 8, D], BF)
            nc.scalar.dma_start(out=w2s[:], in_=w2t[:])
            w3s = cpool.tile([128, 2, 4 * D], BF)
            nc.scalar.dma_start(out=w3s[:], in_=w3t[:])
            w4s = cpool.tile([128, 8, D], BF)
            nc.scalar.dma_start(out=w4s[:], in_=w4t[:])
            w5s = cpool.tile([128, 2, D], BF)
            nc.scalar.dma_start(out=w5s[:], in_=w5t[:])

            at_sb = []
            for b in range(BPC):
                t = apool.tile([128, 4, L], BF, tag="at", name=f"at{b}")
                nc.sync.dma_start(out=t[:], in_=atd[b])
                at_sb.append(t)

            bias_sb = []
            for i, dram in enumerate(biasd):
                if dram is None:
                    bias_sb.append(None)
                else:
                    t = cpool.tile([1, bdims[i]], BF, tag=f"bias{i}")
                    nc.scalar.dma_start(out=t[:], in_=dram[:])
                    bias_sb.append(t)
            any_bias = any(bias_flags)
            if any_bias:
                ones = cpool.tile([1, L], BF)
                nc.vector.memset(ones[:], 1.0)

            # ---- PE p-state warmup: dead matmuls while first DMAs land ----
            wu = cpool.tile([128, L], BF)
            nc.vector.memset(wu[:], 0.0)
            wups = ppool.tile([128, L], F32, tag="mm")
            for _ in range(6):
                nc.tensor.matmul(out=wups[:], lhsT=wu[:, 0:128], rhs=wu[:],
                                 start=True, stop=True)

            # ---- emb^T = tabs^T @ MH (+pe on the PSUM->SBUF move) ----
            # x[b] bf16 [128(d), 2(dc), 512(l)] — the MLP input AND the
            # emb term of the output (kept resident until the end).
            xs = []
            for b in range(BPC):
                x = xpool.tile([128, 2, L], BF, tag="x")
                for dc in range(2):
                    ps = ppool.tile([128, L], F32, tag="mm")
                    for vc in range(2):
                        nc.tensor.matmul(
                            out=ps[:],
                            lhsT=tabs_sb[:, vc, dc * 128:(dc + 1) * 128],
                            rhs=mh_sb[b][:, vc, :],
                            start=(vc == 0), stop=(vc == 1))
                    nc.vector.tensor_tensor(
                        out=x[:, dc, :], in0=ps[:], in1=pedl_sb[:, dc, :],
                        op=ALU.add)
                xs.append(x)

            def relu_out(dst, ps, i):
                # split relus across ACT and DVE (ACT gets the larger share)
                if i % 8 in (0, 3):
                    nc.vector.tensor_scalar(
                        out=dst, in0=ps[:], scalar1=0.0, scalar2=None,
                        op0=ALU.max)
                else:
                    nc.scalar.activation(out=dst, in_=ps[:], func=AF.Relu)

            # expand layer: y = relu(w x + b), weight-stationary over b
            def expand(xin_list, wsb, bsb, ytag):
                youts = [ypool.tile([128, 8, L], BF, tag=ytag, name=f"y{b}")
                         for b in range(BPC)]
                for m in range(8):
                    pss = []
                    for b in range(BPC):
                        ps = ppool.tile([128, L], F32, tag="mm")
                        for kc in range(2):
                            nc.tensor.matmul(
                                out=ps[:],
                                lhsT=wsb[:, kc, m * 128:(m + 1) * 128],
                                rhs=xin_list[b][:, kc, :],
                                start=(kc == 0),
                                stop=(kc == 1 and bsb is None))
                        if bsb is not None:
                            nc.tensor.matmul(
                                out=ps[:],
                                lhsT=bsb[:1, m * 128:(m + 1) * 128],
                                rhs=ones[:1, :],
                                start=False, stop=True)
                        pss.append(ps)
                    for b in range(BPC):
                        relu_out(youts[b][:, m, :], pss[b], m * BPC + b)
                return youts

            # contract layer: xnew = xres + w y + b, weight-stationary over b
            def contract(y_list, wsb, bsb, xres_list):
                xouts = [xpool.tile([128, 2, L], BF, tag="x", name=f"xn{b}")
                         for b in range(BPC)]
                for m in range(2):
                    pss = []
                    for b in range(BPC):
                        ps = ppool.tile([128, L], F32, tag="mm")
                        for kc in range(8):
                            nc.tensor.matmul(
                                out=ps[:],
                                lhsT=wsb[:, kc, m * 128:(m + 1) * 128],
                                rhs=y_list[b][:, kc, :],
                                start=(kc == 0),
                                stop=(kc == 7 and bsb is None))
                        if bsb is not None:
                            nc.tensor.matmul(
                                out=ps[:],
                                lhsT=bsb[:1, m * 128:(m + 1) * 128],
                                rhs=ones[:1, :],
                                start=False, stop=True)
                        pss.append(ps)
                    for b in range(BPC):
                        nc.vector.tensor_tensor(
                            out=xouts[b][:, m, :], in0=pss[b][:],
                            in1=xres_list[b][:, m, :], op=ALU.add)
                return xouts

            y1s = expand(xs, w1s, bias_sb[0], "y")
            x1s = contract(y1s, w2s, bias_sb[1], xs)
            y3s = expand(x1s, w3s, bias_sb[2], "y")
            x2s = contract(y3s, w4s, bias_sb[3], x1s)

            # head (transposed): msgT[l, d] = x2^T @ W5^T; then
            # aggT[d, l] = msgT^T @ A^T; out^T = aggT + x.
            def head(b):
                msgT = mpool.tile([128, 4, D], BF, tag="msgT")
                for jc in range(4):
                    ps = p5pool.tile([128, D], F32, tag="p5")
                    for dc in range(2):
                        nc.tensor.matmul(
                            out=ps[:],
                            lhsT=x2s[b][:, dc, jc * 128:(jc + 1) * 128],
                            rhs=w5s[:, dc, :],
                            start=(dc == 0),
                            stop=(dc == 1 and bias_sb[4] is None))
                    if bias_sb[4] is not None:
                        nc.tensor.matmul(
                            out=ps[:],
                            lhsT=ones[:1, jc * 128:(jc + 1) * 128],
                            rhs=bias_sb[4][:1, :],
                            start=False, stop=True)
                    if jc % 2:
                        nc.vector.tensor_copy(out=msgT[:, jc, :], in_=ps[:])
                    else:
                        nc.scalar.activation(out=msgT[:, jc, :], in_=ps[:],
                                             func=AF.Copy)
                return msgT

            def agg(b, msgT):
                ot = opool.tile([128, 2, L], BF, tag="ot")
                for dc in range(2):
                    ps = ppool.tile([128, L], F32, tag="mm")
                    for jc in range(4):
                        nc.tensor.matmul(
                            out=ps[:],
                            lhsT=msgT[:, jc, dc * 128:(dc + 1) * 128],
                            rhs=at_sb[b][:, jc, :],
                            start=(jc == 0), stop=(jc == 3))
                    nc.vector.tensor_tensor(
                        out=ot[:, dc, :], in0=ps[:], in1=xs[b][:, dc, :],
                        op=ALU.add)
                    # store each half as soon as it's ready (shorter tail)
                    nc.sync.dma_start(out=outp[b, dc], in_=ot[:, dc, :])

            # software-pipeline the head: msgT[b+1] copies overlap aggT[b]
            msgTs = [head(0), head(1)]
            agg(0, msgTs[0])
            msgTs.append(head(2))
            agg(1, msgTs[1])
            msgTs.append(head(3))
            agg(2, msgTs[2])
            agg(3, msgTs[3])

    nc.compile()
    _prog_cache[key] = nc
    return nc


def _host_prep(inp):
    """Build per-core in_maps."""
    element = np.asarray(inp["element"]).astype(np.int64)
    bond = np.asarray(inp["bond"]).astype(np.int64)
    aroma = np.asarray(inp["aroma"]).astype(np.int64)
    charge = np.asarray(inp["charge"]).astype(np.int64)
    segment = np.asarray(inp["segment"]).astype(np.int64)
    react = np.asarray(inp["reactant_mask"]).astype(np.int64)

    tab = np.zeros((VCAT, D), np.float32)
    tab[0:128] = np.asarray(inp["elem_emb"])
    ar = np.asarray(inp["aroma_emb"], dtype=np.float32)
    ch = np.asarray(inp["charge_emb"], dtype=np.float32)
    sg = np.asarray(inp["seg_emb"], dtype=np.float32)
    rc = np.asarray(inp["react_emb"], dtype=np.float32)
    tab[OFF_AC:OFF_AC + 26] = (ar[:, None, :] + ch[None, :, :]).reshape(26, D)
    tab[OFF_SR:OFF_SR + 60] = (sg[:, None, :] + rc[None, :, :]).reshape(60, D)
    tabs_host = np.ascontiguousarray(
        tab.reshape(2, 128, D).transpose(1, 0, 2)).astype(BF16)

    pe = np.asarray(inp["pe"]).reshape(L, D).astype(np.float32)
    pedl_host = np.ascontiguousarray(
        pe.T.reshape(2, 128, L).transpose(1, 0, 2)).astype(BF16)

    def wprep(w, kchunks):  # w [dout, din] -> [128, kchunks, dout] bf16
        wT = np.asarray(w).T  # [din, dout]
        return np.ascontiguousarray(
            wT.reshape(kchunks, 128, wT.shape[1]).transpose(1, 0, 2)).astype(BF16)

    w1t = wprep(inp["w1"], 2)
    w2t = wprep(inp["w2"], 8)
    w3t = wprep(inp["w3"], 2)
    w4t = wprep(inp["w4"], 8)
    w5t = wprep(inp["w5"], 2)

    biases = [np.asarray(inp[f"b{i}"]).astype(np.float32) for i in range(1, 6)]
    bias_flags = tuple(bool(np.any(b != 0.0)) for b in biases)
    bias_rows = [b.reshape(1, -1).astype(BF16) for b in biases]

    # multi-hot [B, VCAT, L]: 3 disjoint one rows per token column
    rows = np.stack([
        element,
        OFF_AC + aroma * 13 + (charge + 6),
        OFF_SR + segment * 2 + react,
    ])  # [3, B, L]
    mh = np.zeros((B, VCAT, L), np.float32)
    bidx = np.arange(B)[None, :, None]
    lidx = np.arange(L)[None, None, :]
    mh[bidx, rows, lidx] = 1.0
    mh_host = np.ascontiguousarray(
        mh.reshape(B, 2, 128, L).transpose(0, 2, 1, 3)).astype(BF16)

    # adjacency A^T per molecule
    lr = np.arange(L)
    lrep = np.repeat(lr, K)
    in_maps = []
    for c in range(NCORES):
        at = np.empty((BPC, 128, 4, L), np.float32)
        for bl, bg in enumerate(range(c * BPC, (c + 1) * BPC)):
            A = np.zeros((L, L), np.float32)
            np.add.at(A, (lrep, bond[bg].ravel()), 1.0)
            A[lr, lr] = 0.0
            at[bl] = A.T.reshape(4, 128, L).transpose(1, 0, 2)
        m = {
            "tabs": tabs_host,
            "mh": mh_host[c * BPC:(c + 1) * BPC],
            "pedl": pedl_host,
            "w1t": w1t, "w2t": w2t, "w3t": w3t, "w4t": w4t, "w5t": w5t,
            "at": at.astype(BF16),
        }
        for i, flag in enumerate(bias_flags):
            if flag:
                m[f"bias{i + 1}"] = bias_rows[i]
        in_maps.append(m)
    return in_maps, bias_flags


def kernel(**inputs):
    global LAST_RESULTS
    from concourse.bass_utils import run_bass_kernel_spmd
    in_maps, bias_flags = _host_prep(inputs)
    nc = _build_program(bias_flags)
    trace = bool(int(os.environ.get("ATOM_TRACE", "0")))
    res = run_bass_kernel_spmd(nc, in_maps, list(range(NCORES)), trace=trace)
    LAST_RESULTS = res
    out = np.empty((L, B, D), np.float32)
    for c in range(NCORES):
        o = np.asarray(res.results[c]["out"]).astype(np.float32)
        # o[b, dc, p, l] -> out[l, b, dc*128+p]
        out[:, c * BPC:(c + 1) * BPC, :] = o.transpose(3, 0, 1, 2).reshape(
            L, BPC, D)
    return out


# revision 12
# speedup vs baseline: 1.8292x; 1.0310x over previous
"""Trainium2 Bass kernel for nn_AtomEncoder (gnn_message_passing).

Strategy (8 NeuronCores, data-parallel over batch, 4 molecules/core):
  - embedding lookups as a multi-hot matmul: emb^T[d,l] = tabs^T @ MH with
    MH a host-built 0/1 matrix (3 ones per token column); pe added on the
    PSUM->SBUF move.  No gathers, no transposes, PE starts immediately.
  - MLP (two residual 1x1-conv blocks): bf16 PE matmuls in [d, l]
    orientation, weight-stationary inner ordering across molecules.
  - head computed transposed: msgT[l,d] = x2^T @ W5^T (x2 stationary).
  - bond aggregation transposed: aggT[d,l] = msgT^T @ A^T (A host-built
    dense one-hot adjacency, diagonal zeroed); out^T = aggT + x via DVE.
  - output stored [d, l] bf16 in 4 contiguous blocks, unshuffled on host.
"""

import os
import sys
import types

sys.path.insert(0, "/opt/trn_rl_repo")

import numpy as np
import ml_dtypes

BF16 = ml_dtypes.bfloat16
FP8 = ml_dtypes.float8_e4m3

B, L, D, K, NCORES = 32, 512, 256, 6, 8
BPC = B // NCORES          # batch elements per core
NTOK = 128                 # element vocab
# concatenated table offsets: [elem(128), aroma x charge(26), seg x react(60)]
OFF_AC, OFF_SR = 128, 154
VCAT = 256                 # padded concat table rows (2 partition chunks)

LAST_RESULTS = None        # BassKernelResults of the most recent run (for test.py)


def _install_ntff_hook():
    """The agent image lacks antenv.axon_hooks; synthesize it so
    run_bass_kernel_spmd(trace=True) can profile via NTFF."""
    try:
        from antenv.axon_hooks import get_axon_ntff_profile_hook  # noqa: F401
        return
    except ImportError:
        pass
    try:
        import antenv
    except ImportError:
        return
    m = types.ModuleType("antenv.axon_hooks")
    m._hook = None
    m.set_axon_ntff_profile_hook = lambda h: setattr(m, "_hook", h)
    m.get_axon_ntff_profile_hook = lambda: m._hook
    sys.modules["antenv.axon_hooks"] = m
    antenv.axon_hooks = m
    try:
        if "/root/.axon_site" not in sys.path:
            sys.path.append("/root/.axon_site")
        from trn_agent_boot.trn_boot import _ntff_profile_via_ctypes
        m._hook = _ntff_profile_via_ctypes("/opt/axon/libaxon_pjrt.so")
    except Exception:
        pass


_install_ntff_hook()

import concourse.bacc as bacc           # noqa: E402
import concourse.mybir as mybir         # noqa: E402
import concourse.bass_utils as bass_utils  # noqa: E402
from concourse.tile import TileContext  # noqa: E402

# zero-egress container: skip artifact upload in the trace path
bass_utils.upload_artifacts = lambda tmpdir: f"local:{tmpdir}"

F32 = mybir.dt.float32
BF = mybir.dt.bfloat16
F8 = mybir.dt.float8e4
AF = mybir.ActivationFunctionType
ALU = mybir.AluOpType

_prog_cache = {}


def _build_program(bias_flags):
    """bias_flags: (b1,b2,b3,b4,b5) bools — emit bias matmuls only if nonzero."""
    key = tuple(bias_flags)
    if key in _prog_cache:
        return _prog_cache[key]

    nc = bacc.Bacc("TRN2", target_bir_lowering=False, debug=False,
                   num_devices=NCORES)

    tabsd = nc.dram_tensor("tabs", [128, 2, D], BF, kind="ExternalInput")
    mhd = nc.dram_tensor("mh", [BPC, 128, 2, L], BF, kind="ExternalInput")
    pedld = nc.dram_tensor("pedl", [128, 2, L], BF, kind="ExternalInput")
    w1t = nc.dram_tensor("w1t", [128, 2, 4 * D], F8, kind="ExternalInput")
    w2t = nc.dram_tensor("w2t", [128, 8, D], BF, kind="ExternalInput")
    w3t = nc.dram_tensor("w3t", [128, 2, 4 * D], BF, kind="ExternalInput")
    w4t = nc.dram_tensor("w4t", [128, 8, D], BF, kind="ExternalInput")
    w5t = nc.dram_tensor("w5t", [128, 2, D], BF, kind="ExternalInput")
    atd = nc.dram_tensor("at", [BPC, 128, 4, L], BF, kind="ExternalInput")
    biasd = []
    bdims = [4 * D, D, 4 * D, D, D]
    for i, flag in enumerate(bias_flags):
        biasd.append(
            nc.dram_tensor(f"bias{i + 1}", [1, bdims[i]], BF, kind="ExternalInput")
            if flag else None)
    outp = nc.dram_tensor("out", [BPC, 2, 128, L], BF, kind="ExternalOutput")

    with TileContext(nc) as tc:
        with (
            tc.tile_pool(name="const", bufs=1) as cpool,
            tc.tile_pool(name="apool", bufs=4) as apool,
            tc.tile_pool(name="xpool", bufs=12) as xpool,
            tc.tile_pool(name="x8pool", bufs=4) as x8pool,
            tc.tile_pool(name="ypool", bufs=5) as ypool,
            tc.tile_pool(name="mpool", bufs=4) as mpool,
            tc.tile_pool(name="opool", bufs=4) as opool,
            tc.tile_pool(name="psum", bufs=6, space="PSUM") as ppool,
            tc.tile_pool(name="psum5", bufs=2, space="PSUM") as p5pool,
        ):
            # ---- loads in need-order on the two HW-DGE queues ----
            # Each queue is FIFO (priority = program order) and the two
            # queues split HBM bandwidth, so balance bytes and interleave
            # by first-use time.  Software DGE (gpsimd) unused — it
            # contends for HBM and adds a long queue-drain tail.
            # sync:   tabs, mh0, mh2, mh3, at0-3      (~2.7 MB)
            # scalar: mh1, pedl, w1..w5, biases       (~2.7 MB)
            tabs_sb = cpool.tile([128, 2, D], BF)
            nc.sync.dma_start(out=tabs_sb[:], in_=tabsd[:])
            mh_sb = [cpool.tile([128, 2, L], BF, tag=f"mh{b}", name=f"mh{b}")
                     for b in range(BPC)]
            nc.sync.dma_start(out=mh_sb[0][:], in_=mhd[0])
            nc.scalar.dma_start(out=mh_sb[1][:], in_=mhd[1])
            pedl_sb = cpool.tile([128, 2, L], BF)
            nc.scalar.dma_start(out=pedl_sb[:], in_=pedld[:])
            nc.sync.dma_start(out=mh_sb[2][:], in_=mhd[2])
            nc.sync.dma_start(out=mh_sb[3][:], in_=mhd[3])

            w1s = cpool.tile([128, 2, 4 * D], F8)
            nc.scalar.dma_start(out=w1s[:], in_=w1t[:])
            w2s = cpool.tile([128, 8, D], BF)
            nc.scalar.dma_start(out=w2s[:], in_=w2t[:])
            w3s = cpool.tile([128, 2, 4 * D], BF)
            nc.scalar.dma_start(out=w3s[:], in_=w3t[:])
            w4s = cpool.tile([128, 8, D], BF)
            nc.scalar.dma_start(out=w4s[:], in_=w4t[:])
            w5s = cpool.tile([128, 2, D], BF)
            nc.scalar.dma_start(out=w5s[:], in_=w5t[:])

            at_sb = []
            for b in range(BPC):
                t = apool.tile([128, 4, L], BF, tag="at", name=f"at{b}")
                nc.sync.dma_start(out=t[:], in_=atd[b])
                at_sb.append(t)

            bias_sb = []
            for i, dram in enumerate(biasd):
                if dram is None:
                    bias_sb.append(None)
                else:
                    t = cpool.tile([1, bdims[i]], BF, tag=f"bias{i}")
                    nc.scalar.dma_start(out=t[:], in_=dram[:])
                    bias_sb.append(t)
            any_bias = any(bias_flags)
            if any_bias:
                ones = cpool.tile([1, L], BF)
                nc.vector.memset(ones[:], 1.0)

            # ---- PE p-state warmup: dead matmuls while first DMAs land ----
            wu = cpool.tile([128, L], BF)
            nc.vector.memset(wu[:], 0.0)
            wups = ppool.tile([128, L], F32, tag="mm")
            for _ in range(9):
                nc.tensor.matmul(out=wups[:], lhsT=wu[:, 0:128], rhs=wu[:],
                                 start=True, stop=True)

            # ---- emb^T = tabs^T @ MH (+pe on the PSUM->SBUF move) ----
            # x[b] bf16 [128(d), 2(dc), 512(l)] — the MLP input AND the
            # emb term of the output (kept resident until the end).
            xs = []
            x8s = []
            for b in range(BPC):
                x = xpool.tile([128, 2, L], BF, tag="x")
                x8 = x8pool.tile([128, 2, L], F8, tag="x8")
                for dc in range(2):
                    ps = ppool.tile([128, L], F32, tag="mm")
                    for vc in range(2):
                        nc.tensor.matmul(
                            out=ps[:],
                            lhsT=tabs_sb[:, vc, dc * 128:(dc + 1) * 128],
                            rhs=mh_sb[b][:, vc, :],
                            start=(vc == 0), stop=(vc == 1))
                    nc.vector.tensor_tensor(
                        out=x[:, dc, :], in0=ps[:], in1=pedl_sb[:, dc, :],
                        op=ALU.add)
                # fp8 copy of x for the DoubleRow L1 input
                nc.scalar.activation(out=x8[:], in_=x[:], func=AF.Copy)
                xs.append(x)
                x8s.append(x8)

            def relu_out(dst, ps, i):
                # split relus across ACT and DVE (ACT gets the larger share)
                if i % 8 in (0, 3):
                    nc.vector.tensor_scalar(
                        out=dst, in0=ps[:], scalar1=0.0, scalar2=None,
                        op0=ALU.max)
                else:
                    nc.scalar.activation(out=dst, in_=ps[:], func=AF.Relu)

            # expand layer (fp8 DoubleRow): y = relu(w x + b)
            def expand8(xin_list, wsb, bsb, ytag):
                youts = [ypool.tile([128, 8, L], BF, tag=ytag, name=f"y8{b}")
                         for b in range(BPC)]
                for m in range(8):
                    pss = []
                    for b in range(BPC):
                        ps = ppool.tile([128, L], F32, tag="mm")
                        nc.tensor.matmul(
                            out=ps[:],
                            lhsT=wsb[:, :, m * 128:(m + 1) * 128],
                            rhs=xin_list[b][:],
                            perf_mode=mybir.MatmulPerfMode.DoubleRow,
                            start=True, stop=(bsb is None))
                        if bsb is not None:
                            nc.tensor.matmul(
                                out=ps[:],
                                lhsT=bsb[:1, m * 128:(m + 1) * 128],
                                rhs=ones[:1, :],
                                start=False, stop=True)
                        pss.append(ps)
                    for b in range(BPC):
                        relu_out(youts[b][:, m, :], pss[b], m * BPC + b)
                return youts

            # expand layer: y = relu(w x + b), weight-stationary over b
            def expand(xin_list, wsb, bsb, ytag):
                youts = [ypool.tile([128, 8, L], BF, tag=ytag, name=f"y{b}")
                         for b in range(BPC)]
                for m in range(8):
                    pss = []
                    for b in range(BPC):
                        ps = ppool.tile([128, L], F32, tag="mm")
                        for kc in range(2):
                            nc.tensor.matmul(
                                out=ps[:],
                                lhsT=wsb[:, kc, m * 128:(m + 1) * 128],
                                rhs=xin_list[b][:, kc, :],
                                start=(kc == 0),
                                stop=(kc == 1 and bsb is None))
                        if bsb is not None:
                            nc.tensor.matmul(
                                out=ps[:],
                                lhsT=bsb[:1, m * 128:(m + 1) * 128],
                                rhs=ones[:1, :],
                                start=False, stop=True)
                        pss.append(ps)
                    for b in range(BPC):
                        relu_out(youts[b][:, m, :], pss[b], m * BPC + b)
                return youts

            # contract layer: xnew = xres + w y + b, weight-stationary over b
            def contract(y_list, wsb, bsb, xres_list):
                xouts = [xpool.tile([128, 2, L], BF, tag="x", name=f"xn{b}")
                         for b in range(BPC)]
                for m in range(2):
                    pss = []
                    for b in range(BPC):
                        ps = ppool.tile([128, L], F32, tag="mm")
                        for kc in range(8):
                            nc.tensor.matmul(
                                out=ps[:],
                                lhsT=wsb[:, kc, m * 128:(m + 1) * 128],
                                rhs=y_list[b][:, kc, :],
                                start=(kc == 0),
                                stop=(kc == 7 and bsb is None))
                        if bsb is not None:
                            nc.tensor.matmul(
                                out=ps[:],
                                lhsT=bsb[:1, m * 128:(m + 1) * 128],
                                rhs=ones[:1, :],
                                start=False, stop=True)
                        pss.append(ps)
                    for b in range(BPC):
                        nc.vector.tensor_tensor(
                            out=xouts[b][:, m, :], in0=pss[b][:],
                            in1=xres_list[b][:, m, :], op=ALU.add)
                return xouts

            y1s = expand8(x8s, w1s, bias_sb[0], "y")
            x1s = contract(y1s, w2s, bias_sb[1], xs)
            y3s = expand(x1s, w3s, bias_sb[2], "y")
            x2s = contract(y3s, w4s, bias_sb[3], x1s)

            # head (transposed): msgT[l, d] = x2^T @ W5^T; then
            # aggT[d, l] = msgT^T @ A^T; out^T = aggT + x.
            def head(b):
                msgT = mpool.tile([128, 4, D], BF, tag="msgT")
                for jc in range(4):
                    ps = p5pool.tile([128, D], F32, tag="p5")
                    for dc in range(2):
                        nc.tensor.matmul(
                            out=ps[:],
                            lhsT=x2s[b][:, dc, jc * 128:(jc + 1) * 128],
                            rhs=w5s[:, dc, :],
                            start=(dc == 0),
                            stop=(dc == 1 and bias_sb[4] is None))
                    if bias_sb[4] is not None:
                        nc.tensor.matmul(
                            out=ps[:],
                            lhsT=ones[:1, jc * 128:(jc + 1) * 128],
                            rhs=bias_sb[4][:1, :],
                            start=False, stop=True)
                    if jc % 2:
                        nc.vector.tensor_copy(out=msgT[:, jc, :], in_=ps[:])
                    else:
                        nc.scalar.activation(out=msgT[:, jc, :], in_=ps[:],
                                             func=AF.Copy)
                return msgT

            def agg(b, msgT):
                ot = opool.tile([128, 2, L], BF, tag="ot")
                for dc in range(2):
                    ps = ppool.tile([128, L], F32, tag="mm")
                    for jc in range(4):
                        nc.tensor.matmul(
                            out=ps[:],
                            lhsT=msgT[:, jc, dc * 128:(dc + 1) * 128],
                            rhs=at_sb[b][:, jc, :],
                            start=(jc == 0), stop=(jc == 3))
                    nc.vector.tensor_tensor(
                        out=ot[:, dc, :], in0=ps[:], in1=xs[b][:, dc, :],
                        op=ALU.add)
                    # store each half as soon as it's ready (shorter tail)
                    nc.sync.dma_start(out=outp[b, dc], in_=ot[:, dc, :])

            # software-pipeline the head: msgT[b+1] copies overlap aggT[b]
            msgTs = [head(0), head(1)]
            agg(0, msgTs[0])
            msgTs.append(head(2))
            agg(1, msgTs[1])
            msgTs.append(head(3))
            agg(2, msgTs[2])
            agg(3, msgTs[3])

    nc.compile()
    _prog_cache[key] = nc
    return nc


def _host_prep(inp):
    """Build per-core in_maps."""
    element = np.asarray(inp["element"]).astype(np.int64)
    bond = np.asarray(inp["bond"]).astype(np.int64)
    aroma = np.asarray(inp["aroma"]).astype(np.int64)
    charge = np.asarray(inp["charge"]).astype(np.int64)
    segment = np.asarray(inp["segment"]).astype(np.int64)
    react = np.asarray(inp["reactant_mask"]).astype(np.int64)

    tab = np.zeros((VCAT, D), np.float32)
    tab[0:128] = np.asarray(inp["elem_emb"])
    ar = np.asarray(inp["aroma_emb"], dtype=np.float32)
    ch = np.asarray(inp["charge_emb"], dtype=np.float32)
    sg = np.asarray(inp["seg_emb"], dtype=np.float32)
    rc = np.asarray(inp["react_emb"], dtype=np.float32)
    tab[OFF_AC:OFF_AC + 26] = (ar[:, None, :] + ch[None, :, :]).reshape(26, D)
    tab[OFF_SR:OFF_SR + 60] = (sg[:, None, :] + rc[None, :, :]).reshape(60, D)
    tabs_host = np.ascontiguousarray(
        tab.reshape(2, 128, D).transpose(1, 0, 2)).astype(BF16)

    pe = np.asarray(inp["pe"]).reshape(L, D).astype(np.float32)
    pedl_host = np.ascontiguousarray(
        pe.T.reshape(2, 128, L).transpose(1, 0, 2)).astype(BF16)

    def wprep(w, kchunks):  # w [dout, din] -> [128, kchunks, dout] bf16
        wT = np.asarray(w).T  # [din, dout]
        return np.ascontiguousarray(
            wT.reshape(kchunks, 128, wT.shape[1]).transpose(1, 0, 2)).astype(BF16)

    w1t = wprep(inp["w1"], 2).astype(FP8)
    w2t = wprep(inp["w2"], 8)
    w3t = wprep(inp["w3"], 2)
    w4t = wprep(inp["w4"], 8)
    w5t = wprep(inp["w5"], 2)

    biases = [np.asarray(inp[f"b{i}"]).astype(np.float32) for i in range(1, 6)]
    bias_flags = tuple(bool(np.any(b != 0.0)) for b in biases)
    bias_rows = [b.reshape(1, -1).astype(BF16) for b in biases]

    # multi-hot [B, VCAT, L]: 3 disjoint one rows per token column
    rows = np.stack([
        element,
        OFF_AC + aroma * 13 + (charge + 6),
        OFF_SR + segment * 2 + react,
    ])  # [3, B, L]
    mh = np.zeros((B, VCAT, L), np.float32)
    bidx = np.arange(B)[None, :, None]
    lidx = np.arange(L)[None, None, :]
    mh[bidx, rows, lidx] = 1.0
    mh_host = np.ascontiguousarray(
        mh.reshape(B, 2, 128, L).transpose(0, 2, 1, 3)).astype(BF16)

    # adjacency A^T per molecule
    lr = np.arange(L)
    lrep = np.repeat(lr, K)
    in_maps = []
    for c in range(NCORES):
        at = np.empty((BPC, 128, 4, L), np.float32)
        for bl, bg in enumerate(range(c * BPC, (c + 1) * BPC)):
            A = np.zeros((L, L), np.float32)
            np.add.at(A, (lrep, bond[bg].ravel()), 1.0)
            A[lr, lr] = 0.0
            at[bl] = A.T.reshape(4, 128, L).transpose(1, 0, 2)
        m = {
            "tabs": tabs_host,
            "mh": mh_host[c * BPC:(c + 1) * BPC],
            "pedl": pedl_host,
            "w1t": w1t, "w2t": w2t, "w3t": w3t, "w4t": w4t, "w5t": w5t,
            "at": at.astype(BF16),
        }
        for i, flag in enumerate(bias_flags):
            if flag:
                m[f"bias{i + 1}"] = bias_rows[i]
        in_maps.append(m)
    return in_maps, bias_flags


def kernel(**inputs):
    global LAST_RESULTS
    from concourse.bass_utils import run_bass_kernel_spmd
    in_maps, bias_flags = _host_prep(inputs)
    nc = _build_program(bias_flags)
    trace = bool(int(os.environ.get("ATOM_TRACE", "0")))
    res = run_bass_kernel_spmd(nc, in_maps, list(range(NCORES)), trace=trace)
    LAST_RESULTS = res
    out = np.empty((L, B, D), np.float32)
    for c in range(NCORES):
        o = np.asarray(res.results[c]["out"]).astype(np.float32)
        # o[b, dc, p, l] -> out[l, b, dc*128+p]
        out[:, c * BPC:(c + 1) * BPC, :] = o.transpose(3, 0, 1, 2).reshape(
            L, BPC, D)
    return out
